# revision 10
# baseline (speedup 1.0000x reference)
"""YOLO anchor-box decode (predictTransform) as a Trainium2 Bass/Tile kernel.

Input : prediction [32, 255, 76, 76] f32, anchors [3,2] f32, inputDim, numClasses
Output: [32, 17328, 85] f32   (decoded boxes in input-image pixel units)

Math per batch (stride = inputDim // 76 = 8, attrs = 85, A = 3 anchors):
  view [255, 5776] -> transpose -> [5776, 255] rows g = (gy*76+gx), cols (a, k)
  k=0: (sigmoid(x) + gx) * stride      k=1: (sigmoid(y) + gy) * stride
  k=2: exp(w) * anchors[a,0]           k=3: exp(h) * anchors[a,1]
  k>=4: sigmoid(.)

Distribution: pure data parallel over batch, 4 batches per core on 8 cores.

Per-core dataflow (memory-bound, ~47 MB traffic/core, HBM ~358 GB/s):
  1. DMA each batch's [255, 5776] into SBUF as [128, 5776] + [127, 5776],
     in 16-partition-row strips: HWDGE HBM->SBUF assigns descriptor i to
     SDMA slot (i mod 32) and slots 16..31 alias onto engine 0, so ops
     must carry <=16 descriptors to use all 16 engines.  Strips are also
     few and large (370 KB): every HWDGE op pays ~2 us of completion
     latency through an 8-deep completion-semaphore window, so op COUNT
     is the DMA-side currency.
  2. TensorE transposes 128x128 fp32 blocks (identity matmul) into PSUM,
     4 g-blocks per PSUM group tile (256-col stride keeps each matmul
     output inside one PSUM bank), pspool bufs=4.
  3. ScalarE reads PSUM and writes SBUF output tiles applying tanh(x/2)
     (whole tile) and exp (w/h cols -> small staging tile).  tanh instead
     of sigmoid keeps every ACT op inside the single `exp_and_others`
     table set: sigmoid and exp live in different ACT table sets and each
     switch costs ~2.7 us.
  4. VectorE: sigmoid = 0.5*tanh + 0.5 (one fused mul-add pass), the x/y
     affine (x stride, + stride*grid offset from a precomputed table) and
     the w/h anchor multiply.
  5. Output tiles span 4 PSUM groups (16 g-blocks, 2 MB) so each batch
     stores in ~4 big DMAs; HBM side is contiguous per g row.
"""

import os

import numpy as np

import concourse.bacc as bacc
import concourse.bass_utils as bass_utils
import concourse.mybir as mybir
import concourse.tile as tile

F32 = mybir.dt.float32

B, CH, G, G2, A, ATT = 32, 255, 76, 5776, 3, 85
NCORES, BPC = 8, 4            # cores, batches per core
TAIL = G2 - 45 * 128          # 16 leftover grid cells per batch
PSTRIDE = 256                 # per-block PSUM column stride (bank-safe for 255 cols)
# Output-tile spans: (first block, [PSUM group block counts])
OSPANS = [(0, [4, 4, 4, 4]), (16, [4, 4, 4, 4]), (32, [4, 4, 4, 2])]

_PROGRAMS = {}
LAST_RESULTS = None


def _build_program(stride: float):
    nc = bacc.Bacc(
        "TRN2",
        target_bir_lowering=False,
        debug=False,
        enable_asserts=False,
        num_devices=NCORES,
    )
    pred = nc.dram_tensor("pred", [BPC, CH, G2], F32, kind="ExternalInput").ap()
    gxy = nc.dram_tensor("gxy", [128, 46 * 6], F32, kind="ExternalInput").ap()
    ancf = nc.dram_tensor("ancf", [128, 24], F32, kind="ExternalInput").ap()
    ident = nc.dram_tensor("ident", [128, 128], F32, kind="ExternalInput").ap()
    out = nc.dram_tensor("out", [BPC, G2 * A, ATT], F32, kind="ExternalOutput").ap()
    out_flat = out.rearrange("b r k -> b (r k)")

    with tile.TileContext(nc) as tc:
        with (
            tc.tile_pool(name="consts", bufs=1) as consts,
            tc.tile_pool(name="inpool", bufs=2) as inpool,
            tc.tile_pool(name="outpool", bufs=3) as outpool,
            tc.tile_pool(name="whpool", bufs=4) as whpool,
            tc.tile_pool(name="pspool", bufs=4, space="PSUM") as pspool,
        ):
            ident_t = consts.tile([128, 128], F32)
            nc.sync.dma_start(out=ident_t, in_=ident)
            gxy_t = consts.tile([128, 46 * 6], F32)
            nc.sync.dma_start(out=gxy_t, in_=gxy)
            ancf_t = consts.tile([128, 24], F32)
            nc.sync.dma_start(out=ancf_t, in_=ancf)

            for b in range(BPC):
                in0 = inpool.tile([128, G2], F32, tag="in0")
                in1 = inpool.tile([127, G2], F32, tag="in1")
                for p0 in range(0, 128, 16):
                    nc.sync.dma_start(
                        out=in0[p0 : p0 + 16, :], in_=pred[b, p0 : p0 + 16, :]
                    )
                for p0 in range(0, 127, 16):
                    p1 = min(p0 + 16, 127)
                    nc.sync.dma_start(
                        out=in1[p0:p1, :], in_=pred[b, 128 + p0 : 128 + p1, :]
                    )

                for ob0, gsizes in OSPANS:
                    nblk = sum(gsizes)
                    outt = outpool.tile([128, 16 * 255], F32, tag="outt")
                    j0 = ob0
                    for nb in gsizes:
                        ps = pspool.tile([128, 4 * PSTRIDE], F32, tag="ps")
                        wht = whpool.tile([128, 4 * 6], F32, tag="wht")
                        oco = (j0 - ob0) * 255  # column offset inside outt
                        for jj in range(nb):
                            g0 = (j0 + jj) * 128
                            gcnt = min(128, G2 - g0)
                            po = jj * PSTRIDE
                            if gcnt < 128:
                                # Tail block: the ACT/DVE ops below read all
                                # 128 partitions of this column range; zero
                                # it first (engine ops can't start at
                                # partition 16), then the transposes
                                # overwrite rows 0..gcnt.  Rows >= gcnt are
                                # never stored to DRAM.
                                nc.vector.memset(ps[:, po : po + 255], 0.0)
                            nc.tensor.transpose(
                                ps[0:gcnt, po : po + 128],
                                in0[:, g0 : g0 + gcnt],
                                ident_t,
                            )
                            nc.tensor.transpose(
                                ps[0:gcnt, po + 128 : po + 255],
                                in1[:, g0 : g0 + gcnt],
                                ident_t[0:127, 0:127],
                            )
                        nw = nb * 255
                        ps_v = ps[:, 0 : nb * PSTRIDE].rearrange(
                            "p (j c) -> p j c", c=PSTRIDE
                        )[:, :, 0:255]
                        out_v = outt[:, oco : oco + nw].rearrange(
                            "p (j c) -> p j c", c=255
                        )
                        nc.scalar.activation(
                            out_v, ps_v, mybir.ActivationFunctionType.Tanh, scale=0.5
                        )
                        ps_wh = ps_v.rearrange("p j (a k) -> p j a k", a=A)[
                            :, :, :, 2:4
                        ]
                        wh_v = wht[:, 0 : nb * 6].rearrange(
                            "p (j a k) -> p j a k", a=A, k=2
                        )
                        nc.scalar.activation(
                            wh_v, ps_wh, mybir.ActivationFunctionType.Exp
                        )
                        # sigmoid = 0.5*tanh + 0.5, fused single pass
                        nc.vector.tensor_scalar(
                            out=outt[:, oco : oco + nw],
                            in0=outt[:, oco : oco + nw],
                            scalar1=0.5,
                            scalar2=0.5,
                            op0=mybir.AluOpType.mult,
                            op1=mybir.AluOpType.add,
                        )
                        out4 = out_v.rearrange("p j (a k) -> p j a k", a=A)
                        xy = out4[:, :, :, 0:2]
                        nc.vector.tensor_scalar_mul(xy, xy, float(stride))
                        gxy_v = gxy_t[:, j0 * 6 : (j0 + nb) * 6].rearrange(
                            "p (j a k) -> p j a k", a=A, k=2
                        )
                        nc.vector.tensor_add(xy, xy, gxy_v)
                        whc = out4[:, :, :, 2:4]
                        anc_v = ancf_t[:, 0 : nb * 6].rearrange(
                            "p (j a k) -> p j a k", a=A, k=2
                        )
                        nc.vector.tensor_mul(whc, wh_v, anc_v)
                        j0 += nb

                    nfull = nblk if (ob0 + nblk) * 128 <= G2 else nblk - 1
                    base = ob0 * 128 * 255
                    dst = out_flat[b, base : base + nfull * 128 * 255].rearrange(
                        "(j p c) -> p j c", p=128, c=255
                    )
                    src = outt[:, 0 : nfull * 255].rearrange(
                        "p (j c) -> p j c", c=255
                    )
                    nc.scalar.dma_start(out=dst, in_=src)
                    if nfull != nblk:
                        tb = base + nfull * 128 * 255
                        dst_t = out_flat[b, tb : tb + TAIL * 255].rearrange(
                            "(p c) -> p c", c=255
                        )
                        nc.scalar.dma_start(
                            out=dst_t,
                            in_=outt[0:TAIL, nfull * 255 : (nfull + 1) * 255],
                        )
    nc.compile()
    return nc


def _tables(stride: float, anchors: np.ndarray):
    g = np.arange(46 * 128, dtype=np.int64)
    gx = (g % G).astype(np.float32) * stride
    gy = (g // G).astype(np.float32) * stride
    gx[g >= G2] = 0.0
    gy[g >= G2] = 0.0
    gxy = np.stack([gx.reshape(46, 128).T, gy.reshape(46, 128).T], axis=-1)
    gxy = np.repeat(gxy[:, :, None, :], A, axis=2)  # [128, 46, 3, 2]
    gxy = np.ascontiguousarray(gxy.reshape(128, 46 * 6), dtype=np.float32)
    ancf = np.ascontiguousarray(
        np.broadcast_to(
            anchors.astype(np.float32)[None, None], (128, 4, A, 2)
        ).reshape(128, 24)
    )
    ident = np.eye(128, dtype=np.float32)
    return gxy, ancf, ident


def get_program(stride: float):
    key = float(stride)
    if key not in _PROGRAMS:
        _PROGRAMS[key] = _build_program(key)
    return _PROGRAMS[key]


def core_inputs(prediction, anchors, inputDim):
    """Host-side prep: per-core input dicts (exposed for testing)."""
    pred = np.asarray(prediction, dtype=np.float32)
    anc = np.asarray(anchors, dtype=np.float32)
    input_dim = int(np.asarray(inputDim))
    assert pred.shape == (B, CH, G, G), pred.shape
    assert anc.shape == (A, 2), anc.shape
    stride = input_dim // G
    predf = pred.reshape(B, CH, G2)
    gxy, ancf, ident = _tables(float(stride), anc)
    in_maps = [
        {
            "pred": np.ascontiguousarray(predf[i * BPC : (i + 1) * BPC]),
            "gxy": gxy,
            "ancf": ancf,
            "ident": ident,
        }
        for i in range(NCORES)
    ]
    return in_maps, stride


def kernel(prediction, anchors, inputDim, numClasses):
    global LAST_RESULTS
    assert int(np.asarray(numClasses)) == ATT - 5
    in_maps, stride = core_inputs(prediction, anchors, inputDim)
    nc = get_program(float(stride))
    kwargs = {}
    if int(os.environ.get("KERNEL_TRACE", "0")):
        kwargs = dict(trace=True, trace_cores=[0])
    res = bass_utils.run_bass_kernel_spmd(
        nc, in_maps, core_ids=list(range(NCORES)), **kwargs
    )
    LAST_RESULTS = res
    return np.concatenate([r["out"] for r in res.results], axis=0)
